# revision 11
# baseline (speedup 1.0000x reference)
"""Multi-head attention (B=4, S=2048, D=1024, H=16, d_k=64) on 8 TRN2 NeuronCores.

Sharding: batch x head-half grid. Core c handles batch c//2 and head-half c%2
(8 of 16 heads). W_q/W_k/W_v are column-split, W_o row-split (tensor parallel);
the two partial outputs per batch are summed on the host (+bo also host-side).

All matmul operands are bf16 (fp32 PSUM accumulation). Bias adds ride the
PSUM->SBUF eviction on DVE (per-partition tensor_scalar for q/k whose bias
lies along partitions, broadcast tensor_tensor for v) instead of K=1 matmuls.

Emission is a software pipeline over 256 attention iterations (nb, hp, sk)
with lookahead 2: body(i) = [exp(i); scores(i+2); PV(i)] so the in-order PE
queue always has scores(i+2) issued BEFORE PV(i) stalls on exp(i) — ScalarE
exps then run back-to-back. K/V/Q projections, per-pair softmax
normalization (reciprocal + DRAM-bounce partition broadcast + DVE multiply)
and the out-projection of the previous block are pushed onto a filler deque
and popped between iterations (paced 1 per 3, plus a readiness pump that
force-pops producers before the consumer iteration is emitted). Block 0's
attention thereby streams directly behind the K/V projection prologue, and
block 3's normalization runs inside block 3 instead of as a serial tail.
"""

from collections import deque
from contextlib import ExitStack

import numpy as np
import ml_dtypes

import concourse.bass as bass
import concourse.mybir as mybir
import concourse.tile as tile
from concourse import bacc
from concourse.bass_utils import run_bass_kernel_spmd

P = 128
S = 2048
DM = 1024          # d_model
DH = 512           # per-core projected dim (8 heads x 64)
DK = 64
NH = 8             # heads per core
NHP = 4            # head pairs per core
SQB = 512          # Sq block width
NB = S // SQB      # 4 blocks
SKT = S // P       # 16 Sk tiles
DIT = DM // P      # 8 d_in tiles
DST = DH // P      # 4 d_out 128-slices (= head pairs)
T = NB * NHP * SKT # 256 attention iterations

f32 = mybir.dt.float32
bf16 = mybir.dt.bfloat16
EXP = mybir.ActivationFunctionType.Exp
ADD = mybir.AluOpType.add
MULT = mybir.AluOpType.mult
BF = ml_dtypes.bfloat16


def build():
    nc = bacc.Bacc("TRN2", target_bir_lowering=False, debug=False)

    qt = nc.declare_dram_parameter("qt", [DIT, NB, P, SQB], bf16, isOutput=False)
    kt = nc.declare_dram_parameter("kt", [DIT, NB, P, SQB], bf16, isOutput=False)
    vt = nc.declare_dram_parameter("vt", [DIT, NB, P, SQB], bf16, isOutput=False)
    # ds-major weight layouts: the ds=0 quarter arrives first so the first
    # projection group can start ~3.5us in.
    wq = nc.declare_dram_parameter("wq", [DST, P, DIT, P], bf16, isOutput=False)
    wk = nc.declare_dram_parameter("wk", [DST, P, DIT, P], bf16, isOutput=False)
    wv = nc.declare_dram_parameter("wv", [P, DIT, DH], bf16, isOutput=False)
    wo = nc.declare_dram_parameter("wo", [P, NHP, 2, DH], bf16, isOutput=False)
    bqt = nc.declare_dram_parameter("bqt", [P, DST], f32, isOutput=False)
    bkt = nc.declare_dram_parameter("bkt", [P, DST], f32, isOutput=False)
    bv = nc.declare_dram_parameter("bv", [1, DH], f32, isOutput=False)
    out = nc.declare_dram_parameter("out", [S, DM], f32, isOutput=True)

    scr = nc.dram_tensor("scr", [NB, NH, SQB], f32)

    with tile.TileContext(nc) as tc, ExitStack() as ctx:
        const = ctx.enter_context(tc.tile_pool(name="const", bufs=1))
        kT_pool = ctx.enter_context(tc.tile_pool(name="kT", bufs=1))
        vA_pool = ctx.enter_context(tc.tile_pool(name="vA", bufs=1))
        xin_pool = ctx.enter_context(tc.tile_pool(name="xin", bufs=10))
        wkv_pool = ctx.enter_context(tc.tile_pool(name="wkv", bufs=1))

        ps_mm = ctx.enter_context(tc.tile_pool(name="ps_mm", bufs=2, space="PSUM"))
        ps_big = ctx.enter_context(tc.tile_pool(name="ps_big", bufs=2, space="PSUM"))
        ps_attn = ctx.enter_context(tc.tile_pool(name="ps_attn", bufs=2, space="PSUM"))

        # ---- prologue-critical DMAs, most-urgent first ----
        wk_sb = wkv_pool.tile([P, DST, DIT, P], bf16)
        nc.gpsimd.dma_start(out=wk_sb[:, 0], in_=wk[0])

        def load_x(src, skb, tag):
            # kx readers (kgroup ds=0..3) are spread across all of block 0,
            # so kx tiles are fully resident (bufs=32, no slot reuse); vx
            # readers run right after their load -> 16 rotating slots.
            ts = []
            eng = nc.sync if tag == "kx" else nc.gpsimd
            for di in range(DIT):
                t = wkv_pool.tile([P, SQB], bf16, tag=tag,
                                  bufs=(32 if tag == "kx" else 16),
                                  name=f"{tag}{skb}_{di}")
                eng.dma_start(out=t, in_=src[di, skb])
                ts.append(t)
            return ts

        kxs = {0: load_x(kt, 0, "kx")}
        wv_sb = wkv_pool.tile([P, DIT, DH], bf16)
        nc.gpsimd.dma_start(out=wv_sb[:, 0:DIT // 2, :], in_=wv[:, 0:DIT // 2, :])
        nc.gpsimd.dma_start(out=wv_sb[:, DIT // 2:, :], in_=wv[:, DIT // 2:, :])
        vxs = {0: load_x(vt, 0, "vx")}
        bkt_sb = const.tile([P, DST], f32)
        nc.sync.dma_start(out=bkt_sb, in_=bkt[:, :])
        bv_sb = const.tile([P, DH], f32)
        nc.sync.dma_start(out=bv_sb, in_=bv[0, :].partition_broadcast(P))
        ones128 = const.tile([P, NH], bf16)
        nc.vector.memset(ones128, 1.0)
        for i in range(1, DST):
            nc.gpsimd.dma_start(out=wk_sb[:, i], in_=wk[i])
        wq_sb = const.tile([P, DST, DIT, P], bf16)
        qx = {0: []}
        for di in range(DIT):
            t = xin_pool.tile([P, SQB], bf16, tag="xin", name=f"qx0_{di}")
            nc.sync.dma_start(out=t, in_=qt[di, 0])
            qx[0].append(t)
        nc.gpsimd.dma_start(out=wq_sb[:, 0], in_=wq[0])
        for i in range(1, DST):
            nc.gpsimd.dma_start(out=wq_sb[:, i], in_=wq[i])
        bqt_sb = const.tile([P, DST], f32)
        nc.sync.dma_start(out=bqt_sb, in_=bqt[:, :])

        kT = [kT_pool.tile([P, S], bf16, name=f"kT{i}", tag=f"kT{i}")
              for i in range(DST)]
        vA = [vA_pool.tile([P, NH, DK + 1], bf16, name=f"vA{i}", tag=f"vA{i}")
              for i in range(SKT)]

        # late-needed weights (queued behind the prologue stream)
        wo_sb = const.tile([P, NHP, 2, DH], bf16)
        bo_loaded = [False]

        # ---- emitted-state tracking for the scheduler ----
        kg_done = set()    # (skb, ds)
        vg_done = set()    # (skb, j)
        qp_done = set()    # (nb, ds)
        qtiles = {nb: [None] * DST for nb in range(NB)}
        pairs = {nb: [None] * NHP for nb in range(NB)}

        def kgroup(skb, ds):
            def fn():
                ps = ps_mm.tile([P, DH], f32, tag="ps_mm", name=f"psk{skb}_{ds}")
                for di in range(DIT):
                    nc.tensor.matmul(
                        ps, lhsT=wk_sb[:, ds, di, :], rhs=kxs[skb][di],
                        start=(di == 0), stop=(di == DIT - 1))
                nc.vector.tensor_scalar_add(
                    kT[ds][:, skb * SQB:(skb + 1) * SQB], ps,
                    bkt_sb[:, ds:ds + 1])
                kg_done.add((skb, ds))
            return fn

        def vgroup(skb, j):
            def fn():
                skt = skb * (SQB // P) + j
                ps = ps_mm.tile([P, DH], f32, tag="ps_mm", name=f"psv{skb}_{j}")
                for di in range(DIT):
                    nc.tensor.matmul(
                        ps, lhsT=vxs[skb][di][:, j * P:(j + 1) * P],
                        rhs=wv_sb[:, di, :],
                        start=(di == 0), stop=(di == DIT - 1))
                va = vA[skt]
                nc.vector.tensor_copy(va[:, :, DK], ones128)
                nc.vector.tensor_tensor(
                    va[:, :, 0:DK], ps.rearrange("p (h x) -> p h x", x=DK),
                    bv_sb.rearrange("p (h x) -> p h x", x=DK), ADD)
                vg_done.add((skb, j))
            return fn

        def load_kv(skb):
            def fn():
                kxs[skb] = load_x(kt, skb, "kx")
                vxs[skb] = load_x(vt, skb, "vx")
            return fn

        def qload(nb):
            def fn():
                qx[nb] = []
                for di in range(DIT):
                    t = xin_pool.tile([P, SQB], bf16, tag="xin",
                                      name=f"qx{nb}_{di}")
                    nc.sync.dma_start(out=t, in_=qt[di, nb])
                    qx[nb].append(t)
            return fn

        def qgroup(nb, ds):
            def fn():
                ps = ps_mm.tile([P, DH], f32, tag="ps_mm", name=f"psq{nb}_{ds}")
                for di in range(DIT):
                    nc.tensor.matmul(
                        ps, lhsT=wq_sb[:, ds, di, :], rhs=qx[nb][di],
                        start=(di == 0), stop=(di == DIT - 1))
                qtile = qT_pool.tile([P, SQB], bf16, tag="qT", name=f"qT{nb}_{ds}")
                nc.vector.tensor_scalar_add(qtile, ps, bqt_sb[:, ds:ds + 1])
                qtiles[nb][ds] = qtile
                qp_done.add((nb, ds))
            return fn

        qT_pool = ctx.enter_context(tc.tile_pool(name="qT", bufs=8))
        probs_pool = ctx.enter_context(tc.tile_pool(name="probs", bufs=3))
        raw_pool = ctx.enter_context(tc.tile_pool(name="raw", bufs=6))
        pair_pool = ctx.enter_context(tc.tile_pool(name="pair", bufs=8))
        ostg_pool = ctx.enter_context(tc.tile_pool(name="ostg", bufs=2))
        c3_pool = ctx.enter_context(tc.tile_pool(name="c3", bufs=4))
        bc_pool = ctx.enter_context(tc.tile_pool(name="bc", bufs=2))
        ob_pool = ctx.enter_context(tc.tile_pool(name="ob", bufs=2))

        def norm_pair(nb, hp, raw_e, raw_o, c3):
            """Reciprocal of the pair's softmax sums, DRAM bounce, partition
            broadcast, DVE normalize into the [128,512] bf16 pair tile."""
            def fn():
                nc.vector.reciprocal(c3, c3)
                nc.sync.dma_start(out=scr[nb, 2 * hp:2 * hp + 2, :], in_=c3)
                pair = pair_pool.tile([P, SQB], bf16, tag="pair",
                                      name=f"pair{nb}_{hp}")
                pairs[nb][hp] = pair
                bce = bc_pool.tile([DK, SQB], f32, tag="bc", name=f"bce{nb}_{hp}")
                nc.sync.dma_start(
                    out=bce, in_=scr[nb, 2 * hp, :].partition_broadcast(DK))
                nc.vector.tensor_mul(pair[0:DK, :], raw_e[0:DK, :], bce)
                bco = bc_pool.tile([DK, SQB], f32, tag="bc", name=f"bco{nb}_{hp}")
                nc.sync.dma_start(
                    out=bco, in_=scr[nb, 2 * hp + 1, :].partition_broadcast(DK))
                ostg = ostg_pool.tile([DK, SQB], bf16, tag="ostg",
                                      name=f"ostg{nb}_{hp}")
                nc.vector.tensor_mul(ostg, raw_o[0:DK, :], bco)
                nc.sync.dma_start(out=pair[DK:P, :], in_=ostg)
            return fn

        def outproj(nb, sq, nb2):
            def fn():
                if not bo_loaded[0]:
                    bo_loaded[0] = True
                pso = ps_mm.tile([P, DH], f32, tag="ps_mm",
                                 name=f"pso{nb}_{sq}_{nb2}")
                for hp in range(NHP):
                    nc.tensor.matmul(
                        pso, lhsT=pairs[nb][hp][:, sq * P:(sq + 1) * P],
                        rhs=wo_sb[:, hp, nb2, :],
                        start=(hp == 0), stop=(hp == NHP - 1))
                ob = ob_pool.tile([P, DH], f32, tag="ob", name=f"ob{nb}_{sq}_{nb2}")
                nc.vector.tensor_copy(ob, pso)
                nc.gpsimd.dma_start(
                    out=out[nb * SQB + sq * P: nb * SQB + (sq + 1) * P,
                            nb2 * DH:(nb2 + 1) * DH],
                    in_=ob)
            return fn

        # ---- attention iteration bodies ----
        def it_of(i):
            nb, r = divmod(i, NHP * SKT)
            hp, sk = divmod(r, SKT)
            return nb, hp, sk

        ps_of = {}
        pr_of = {}
        pa_of = {}
        fillers = deque()

        def ready_sc(i):
            nb, hp, sk = it_of(i)
            return ((sk // 4, hp) in kg_done) and ((nb, hp) in qp_done)

        def pump():
            assert fillers, "filler deque empty while consumer not ready"
            fillers.popleft()()

        def emit_sc(i):
            nb, hp, sk = it_of(i)
            while not ready_sc(i):
                pump()
            ps = ps_big.tile([P, 2, DH], f32, tag="ps_big", name=f"sc{i}")
            q = qtiles[nb][hp]
            nc.tensor.matmul(
                ps[:, 0, :], lhsT=kT[hp][0:DK, sk * P:(sk + 1) * P],
                rhs=q[0:DK, :], start=True, stop=True)
            nc.tensor.matmul(
                ps[:, 1, :], lhsT=kT[hp][DK:P, sk * P:(sk + 1) * P],
                rhs=q[DK:P, :], start=True, stop=True)
            ps_of[i] = ps

        def emit_exp(i):
            ps = ps_of.pop(i)
            pr = probs_pool.tile([P, 2, DH], bf16, tag="probs", name=f"pr{i}")
            nc.scalar.activation(pr.rearrange("p a b -> p (a b)"),
                                 ps.rearrange("p a b -> p (a b)"),
                                 EXP, scale=0.125)
            pr_of[i] = pr

        def emit_pv(i):
            nb, hp, sk = it_of(i)
            while (sk // 4, sk % 4) not in vg_done:
                pump()
            if sk == 0:
                pa_e = ps_attn.tile([DK + 1, DH], f32, tag="ps_attn",
                                    name=f"pae{nb}_{hp}")
                pa_o = ps_attn.tile([DK + 1, DH], f32, tag="ps_attn",
                                    name=f"pao{nb}_{hp}")
                pa_of[(nb, hp)] = (pa_e, pa_o)
            pa_e, pa_o = pa_of[(nb, hp)]
            pr = pr_of.pop(i)
            nc.tensor.matmul(pa_e, lhsT=vA[sk][:, 2 * hp, :], rhs=pr[:, 0, :],
                             start=(sk == 0), stop=(sk == SKT - 1))
            nc.tensor.matmul(pa_o, lhsT=vA[sk][:, 2 * hp + 1, :], rhs=pr[:, 1, :],
                             start=(sk == 0), stop=(sk == SKT - 1))
            if sk == SKT - 1:
                # pair done: evict attn rows + softmax sums, queue its norm
                c3 = c3_pool.tile([2, SQB], f32, tag="c3", name=f"c3_{nb}_{hp}")
                raws = []
                for pa, h in ((pa_e, 2 * hp), (pa_o, 2 * hp + 1)):
                    raw = raw_pool.tile([DK + 1, SQB], f32, tag="raw",
                                        name=f"raw{nb}_{h}")
                    nc.vector.tensor_copy(raw, pa)
                    nc.sync.dma_start(out=c3[h % 2:h % 2 + 1, :],
                                      in_=raw[DK:DK + 1, :])
                    raws.append(raw)
                del pa_of[(nb, hp)]
                fillers.append(norm_pair(nb, hp, raws[0], raws[1], c3))

        # ---- prologue head: first projection groups inline ----
        kgroup(0, 0)()
        for j in range(SQB // P):
            vgroup(0, j)()
        qgroup(0, 0)()

        # block-0 filler schedule, ordered by first need
        fillers.extend([
            load_kv(1), kgroup(1, 0),
            vgroup(1, 0), vgroup(1, 1), vgroup(1, 2), vgroup(1, 3),
            qgroup(0, 1), kgroup(0, 1), kgroup(1, 1),
            load_kv(2), kgroup(2, 0),
            vgroup(2, 0), vgroup(2, 1), vgroup(2, 2), vgroup(2, 3),
            kgroup(2, 1),
            qgroup(0, 2), kgroup(0, 2), kgroup(1, 2), kgroup(2, 2),
            load_kv(3), kgroup(3, 0),
            vgroup(3, 0), vgroup(3, 1), vgroup(3, 2), vgroup(3, 3),
            kgroup(3, 1), kgroup(3, 2),
            qgroup(0, 3), kgroup(0, 3), kgroup(1, 3), kgroup(2, 3),
            kgroup(3, 3),
        ])

        def late_weights():
            nc.gpsimd.dma_start(out=wo_sb, in_=wo[:, :, :, :])
        fillers.append(late_weights)
        fillers.append(qload(1))
        for ds in range(DST):
            fillers.append(qgroup(1, ds))

        # ---- main software-pipelined loop ----
        emit_sc(0)
        emit_sc(1)
        for i in range(T):
            nb, hp, sk = it_of(i)
            if sk == 0 and hp == 0 and nb >= 1:
                # block boundary: queue prev block's out-proj + next q-proj
                for sq in range(SQB // P):
                    for nb2 in range(2):
                        fillers.append(outproj(nb - 1, sq, nb2))
                if nb + 1 < NB:
                    fillers.append(qload(nb + 1))
                    for ds in range(DST):
                        fillers.append(qgroup(nb + 1, ds))
            emit_exp(i)
            if i + 2 < T:
                emit_sc(i + 2)
            emit_pv(i)
            if i % 3 == 2 and fillers:
                fillers.popleft()()

        # ---- tail: drain remaining fillers, then last block's out-proj ----
        while fillers:
            fillers.popleft()()
        for sq in range(SQB // P):
            for nb2 in range(2):
                outproj(NB - 1, sq, nb2)()

    nc.compile()
    return nc


_NC_CACHE = {}


def _get_nc():
    if "nc" not in _NC_CACHE:
        _NC_CACHE["nc"] = build()
    return _NC_CACHE["nc"]


def _tile_xt(x):
    # [S, DM] -> transpose -> [DIT, NB, P, SQB] with each [P, SQB] contiguous
    xt = np.ascontiguousarray(x.T)                      # [DM, S]
    return np.ascontiguousarray(
        xt.reshape(DIT, P, NB, SQB).transpose(0, 2, 1, 3)).astype(BF)


def _wcol_dsmajor(W, cs):
    # [DM, DH-slice] -> [DST, P, DIT, P] (ds-major stationary layout)
    return np.ascontiguousarray(
        W[:, cs].reshape(DIT, P, DST, P).transpose(2, 1, 0, 3)).astype(BF)


def _shard_inputs(Q, K, V, Wq, bq, Wk, bk, Wv, bv, Wo, bo):
    in_maps = []
    qkvT = {}
    for b in range(4):
        qkvT[b] = (_tile_xt(Q[b]), _tile_xt(K[b]), _tile_xt(V[b]))
    halves = []
    for h in range(2):
        cs = slice(h * DH, (h + 1) * DH)
        halves.append(dict(
            wq=_wcol_dsmajor(Wq, cs),
            wk=_wcol_dsmajor(Wk, cs),
            wv=np.ascontiguousarray(
                Wv[:, cs].reshape(DIT, P, DH).transpose(1, 0, 2)).astype(BF),
            wo=np.ascontiguousarray(
                Wo[cs, :].reshape(NHP, P, 2, DH).transpose(1, 0, 2, 3)).astype(BF),
            bqt=np.ascontiguousarray(
                bq[cs].reshape(DST, P).T).astype(np.float32),
            bkt=np.ascontiguousarray(
                bk[cs].reshape(DST, P).T).astype(np.float32),
            bv=bv[cs].reshape(1, DH).astype(np.float32),
        ))
    for c in range(8):
        b, h = c // 2, c % 2
        qT, kT_, vT = qkvT[b]
        m = dict(qt=qT, kt=kT_, vt=vT)
        m.update(halves[h])
        in_maps.append(m)
    return in_maps


TRACE = False
LAST_RESULT = None


def kernel(**inputs):
    global LAST_RESULT
    inputs = {k: np.asarray(v, dtype=np.float32) for k, v in inputs.items()}
    nc = _get_nc()
    in_maps = _shard_inputs(
        inputs["Q"], inputs["K"], inputs["V"],
        inputs["Wq"], inputs["bq"], inputs["Wk"], inputs["bk"],
        inputs["Wv"], inputs["bv"], inputs["Wo"], inputs["bo"])
    r = run_bass_kernel_spmd(nc, in_maps, core_ids=list(range(8)), trace=TRACE)
    LAST_RESULT = r
    outs = [np.asarray(r.results[c]["out"], dtype=np.float32) for c in range(8)]
    full = np.stack([outs[2 * b] + outs[2 * b + 1] for b in range(4)], axis=0)
    return full + inputs["bo"].reshape(1, 1, DM)


# revision 14
# speedup vs baseline: 1.0676x; 1.0676x over previous
"""Multi-head attention (B=4, S=2048, D=1024, H=16, d_k=64) on 8 TRN2 NeuronCores.

Sharding: batch x head-half grid. Core c handles batch c//2 and head-half c%2
(8 of 16 heads). W_q/W_k/W_v are column-split, W_o row-split (tensor parallel);
the two partial outputs per batch are summed on the host (+bo also host-side).

All matmul operands are bf16 (fp32 PSUM accumulation). Bias adds ride the
PSUM->SBUF eviction on DVE (per-partition tensor_scalar for q/k whose bias
lies along partitions, broadcast tensor_tensor for v) instead of K=1 matmuls.

Emission is a software pipeline over 256 attention iterations (nb, hp, sk)
with lookahead 2: body(i) = [exp(i); scores(i+2); PV(i)] so the in-order PE
queue always has scores(i+2) issued BEFORE PV(i) stalls on exp(i) — ScalarE
exps then run back-to-back. K/V/Q projections, per-pair softmax
normalization (reciprocal + DRAM-bounce partition broadcast + DVE multiply)
and the out-projection of the previous block are pushed onto a filler deque
and popped between iterations (paced 1 per 3, plus a readiness pump that
force-pops producers before the consumer iteration is emitted). Block 0's
attention thereby streams directly behind the K/V projection prologue, and
block 3's normalization runs inside block 3 instead of as a serial tail.
"""

from collections import deque
from contextlib import ExitStack

import numpy as np
import ml_dtypes

import concourse.bass as bass
import concourse.mybir as mybir
import concourse.tile as tile
from concourse import bacc
from concourse.bass_utils import run_bass_kernel_spmd

P = 128
S = 2048
DM = 1024          # d_model
DH = 512           # per-core projected dim (8 heads x 64)
DK = 64
NH = 8             # heads per core
NHP = 4            # head pairs per core
SQB = 512          # Sq block width
NB = S // SQB      # 4 blocks
SKT = S // P       # 16 Sk tiles
DIT = DM // P      # 8 d_in tiles
DST = DH // P      # 4 d_out 128-slices (= head pairs)
T = NB * NHP * SKT # 256 attention iterations

f32 = mybir.dt.float32
bf16 = mybir.dt.bfloat16
EXP = mybir.ActivationFunctionType.Exp
ADD = mybir.AluOpType.add
MULT = mybir.AluOpType.mult
BF = ml_dtypes.bfloat16


def build():
    nc = bacc.Bacc("TRN2", target_bir_lowering=False, debug=False)

    qt = nc.declare_dram_parameter("qt", [DIT, NB, P, SQB], bf16, isOutput=False)
    kt = nc.declare_dram_parameter("kt", [DIT, NB, P, SQB], bf16, isOutput=False)
    vt = nc.declare_dram_parameter("vt", [DIT, NB, P, SQB], bf16, isOutput=False)
    # ds-major weight layouts: the ds=0 quarter arrives first so the first
    # projection group can start ~3.5us in.
    wq = nc.declare_dram_parameter("wq", [DST, P, DIT, P], bf16, isOutput=False)
    wk = nc.declare_dram_parameter("wk", [DST, P, DIT, P], bf16, isOutput=False)
    wv = nc.declare_dram_parameter("wv", [P, DIT, DH], bf16, isOutput=False)
    wo = nc.declare_dram_parameter("wo", [P, NHP, 2, DH], bf16, isOutput=False)
    bqt = nc.declare_dram_parameter("bqt", [P, DST], f32, isOutput=False)
    bkt = nc.declare_dram_parameter("bkt", [P, DST], f32, isOutput=False)
    bv = nc.declare_dram_parameter("bv", [1, DH], f32, isOutput=False)
    out = nc.declare_dram_parameter("out", [S, DM], f32, isOutput=True)

    scr = nc.dram_tensor("scr", [NB, NH, SQB], f32)

    with tile.TileContext(nc) as tc, ExitStack() as ctx:
        const = ctx.enter_context(tc.tile_pool(name="const", bufs=1))
        kT_pool = ctx.enter_context(tc.tile_pool(name="kT", bufs=1))
        vA_pool = ctx.enter_context(tc.tile_pool(name="vA", bufs=1))
        xin_pool = ctx.enter_context(tc.tile_pool(name="xin", bufs=10))
        wkv_pool = ctx.enter_context(tc.tile_pool(name="wkv", bufs=1))

        ps_mm = ctx.enter_context(tc.tile_pool(name="ps_mm", bufs=2, space="PSUM"))
        ps_big = ctx.enter_context(tc.tile_pool(name="ps_big", bufs=2, space="PSUM"))
        ps_attn = ctx.enter_context(tc.tile_pool(name="ps_attn", bufs=2, space="PSUM"))

        # ---- prologue-critical DMAs, most-urgent first ----
        wk_sb = wkv_pool.tile([P, DST, DIT, P], bf16)
        nc.gpsimd.dma_start(out=wk_sb[:, 0], in_=wk[0])

        def load_x(src, skb, tag):
            # kx readers (kgroup ds=0..3) are spread across all of block 0,
            # so kx tiles are fully resident (bufs=32, no slot reuse); vx
            # readers run right after their load -> 16 rotating slots.
            ts = []
            eng = nc.sync if tag == "kx" else nc.gpsimd
            for di in range(DIT):
                t = wkv_pool.tile([P, SQB], bf16, tag=tag,
                                  bufs=(32 if tag == "kx" else 16),
                                  name=f"{tag}{skb}_{di}")
                eng.dma_start(out=t, in_=src[di, skb])
                ts.append(t)
            return ts

        kxs = {0: load_x(kt, 0, "kx")}
        wv_sb = wkv_pool.tile([P, DIT, DH], bf16)
        nc.gpsimd.dma_start(out=wv_sb[:, 0:DIT // 2, :], in_=wv[:, 0:DIT // 2, :])
        nc.gpsimd.dma_start(out=wv_sb[:, DIT // 2:, :], in_=wv[:, DIT // 2:, :])
        vxs = {0: load_x(vt, 0, "vx")}
        bkt_sb = const.tile([P, DST], f32)
        nc.sync.dma_start(out=bkt_sb, in_=bkt[:, :])
        bv_sb = const.tile([P, DH], f32)
        nc.sync.dma_start(out=bv_sb, in_=bv[0, :].partition_broadcast(P))
        ones128 = const.tile([P, NH], bf16)
        nc.vector.memset(ones128, 1.0)
        for i in range(1, DST):
            nc.gpsimd.dma_start(out=wk_sb[:, i], in_=wk[i])
        wq_sb = const.tile([P, DST, DIT, P], bf16)
        qx = {0: []}
        for di in range(DIT):
            t = xin_pool.tile([P, SQB], bf16, tag="xin", name=f"qx0_{di}")
            nc.sync.dma_start(out=t, in_=qt[di, 0])
            qx[0].append(t)
        nc.gpsimd.dma_start(out=wq_sb[:, 0], in_=wq[0])
        for i in range(1, DST):
            nc.gpsimd.dma_start(out=wq_sb[:, i], in_=wq[i])
        bqt_sb = const.tile([P, DST], f32)
        nc.sync.dma_start(out=bqt_sb, in_=bqt[:, :])

        kT = [kT_pool.tile([P, S], bf16, name=f"kT{i}", tag=f"kT{i}")
              for i in range(DST)]
        vA = [vA_pool.tile([P, NH, DK + 1], bf16, name=f"vA{i}", tag=f"vA{i}")
              for i in range(SKT)]

        # late-needed weights (queued behind the prologue stream)
        wo_sb = const.tile([P, NHP, 2, DH], bf16)
        bo_loaded = [False]

        # ---- emitted-state tracking for the scheduler ----
        kg_done = set()    # (skb, ds)
        vg_done = set()    # (skb, j)
        qp_done = set()    # (nb, ds)
        qtiles = {nb: [None] * DST for nb in range(NB)}
        pairs = {nb: [None] * NHP for nb in range(NB)}

        def kgroup(skb, ds):
            def fn():
                ps = ps_mm.tile([P, DH], f32, tag="ps_mm", name=f"psk{skb}_{ds}")
                for di in range(DIT):
                    nc.tensor.matmul(
                        ps, lhsT=wk_sb[:, ds, di, :], rhs=kxs[skb][di],
                        start=(di == 0), stop=(di == DIT - 1))
                nc.vector.tensor_scalar_add(
                    kT[ds][:, skb * SQB:(skb + 1) * SQB], ps,
                    bkt_sb[:, ds:ds + 1])
                kg_done.add((skb, ds))
            return fn

        def vgroup(skb, j):
            def fn():
                skt = skb * (SQB // P) + j
                ps = ps_mm.tile([P, DH], f32, tag="ps_mm", name=f"psv{skb}_{j}")
                for di in range(DIT):
                    nc.tensor.matmul(
                        ps, lhsT=vxs[skb][di][:, j * P:(j + 1) * P],
                        rhs=wv_sb[:, di, :],
                        start=(di == 0), stop=(di == DIT - 1))
                va = vA[skt]
                nc.vector.tensor_copy(va[:, :, DK], ones128)
                nc.vector.tensor_tensor(
                    va[:, :, 0:DK], ps.rearrange("p (h x) -> p h x", x=DK),
                    bv_sb.rearrange("p (h x) -> p h x", x=DK), ADD)
                vg_done.add((skb, j))
            return fn

        def load_kv(skb):
            def fn():
                kxs[skb] = load_x(kt, skb, "kx")
                vxs[skb] = load_x(vt, skb, "vx")
            return fn

        def qload(nb):
            def fn():
                qx[nb] = []
                for di in range(DIT):
                    t = xin_pool.tile([P, SQB], bf16, tag="xin",
                                      name=f"qx{nb}_{di}")
                    nc.sync.dma_start(out=t, in_=qt[di, nb])
                    qx[nb].append(t)
            return fn

        def qgroup(nb, ds):
            def fn():
                ps = ps_mm.tile([P, DH], f32, tag="ps_mm", name=f"psq{nb}_{ds}")
                for di in range(DIT):
                    nc.tensor.matmul(
                        ps, lhsT=wq_sb[:, ds, di, :], rhs=qx[nb][di],
                        start=(di == 0), stop=(di == DIT - 1))
                qtile = qT_pool.tile([P, SQB], bf16, tag="qT", name=f"qT{nb}_{ds}")
                nc.vector.tensor_scalar_add(qtile, ps, bqt_sb[:, ds:ds + 1])
                qtiles[nb][ds] = qtile
                qp_done.add((nb, ds))
            return fn

        qT_pool = ctx.enter_context(tc.tile_pool(name="qT", bufs=8))
        probs_pool = ctx.enter_context(tc.tile_pool(name="probs", bufs=3))
        raw_pool = ctx.enter_context(tc.tile_pool(name="raw", bufs=6))
        pair_pool = ctx.enter_context(tc.tile_pool(name="pair", bufs=8))
        ostg_pool = ctx.enter_context(tc.tile_pool(name="ostg", bufs=2))
        c3_pool = ctx.enter_context(tc.tile_pool(name="c3", bufs=4))
        bc_pool = ctx.enter_context(tc.tile_pool(name="bc", bufs=2))
        ob_pool = ctx.enter_context(tc.tile_pool(name="ob", bufs=4))

        def norm_pair(nb, hp, raw_e, raw_o, c3):
            """Fast reciprocal of the pair's softmax sums, SBUF->SBUF
            partition broadcast, DVE normalize into the [128,512] bf16
            pair tile."""
            def fn():
                nc.vector.reciprocal_approx_fast(c3, c3)
                nc.sync.dma_start(out=scr[nb, 2 * hp:2 * hp + 2, :], in_=c3)
                pair = pair_pool.tile([P, SQB], bf16, tag="pair",
                                      name=f"pair{nb}_{hp}")
                pairs[nb][hp] = pair
                bce = bc_pool.tile([DK, SQB], f32, tag="bc", name=f"bce{nb}_{hp}")
                nc.sync.dma_start(
                    out=bce, in_=scr[nb, 2 * hp, :].partition_broadcast(DK))
                nc.vector.tensor_mul(pair[0:DK, :], raw_e[0:DK, :], bce)
                bco = bc_pool.tile([DK, SQB], f32, tag="bc", name=f"bco{nb}_{hp}")
                nc.sync.dma_start(
                    out=bco, in_=scr[nb, 2 * hp + 1, :].partition_broadcast(DK))
                ostg = ostg_pool.tile([DK, SQB], bf16, tag="ostg",
                                      name=f"ostg{nb}_{hp}")
                nc.vector.tensor_mul(ostg, raw_o[0:DK, :], bco)
                nc.sync.dma_start(out=pair[DK:P, :], in_=ostg)
            return fn

        def outproj(nb, sq, nb2):
            def fn():
                if not bo_loaded[0]:
                    bo_loaded[0] = True
                pso = ps_mm.tile([P, DH], f32, tag="ps_mm",
                                 name=f"pso{nb}_{sq}_{nb2}")
                for hp in range(NHP):
                    nc.tensor.matmul(
                        pso, lhsT=pairs[nb][hp][:, sq * P:(sq + 1) * P],
                        rhs=wo_sb[:, hp, nb2, :],
                        start=(hp == 0), stop=(hp == NHP - 1))
                ob = ob_pool.tile([P, DH], f32, tag="ob", name=f"ob{nb}_{sq}_{nb2}")
                nc.vector.tensor_copy(ob, pso)
                eng = nc.gpsimd if (sq + nb2) % 2 == 0 else nc.sync
                eng.dma_start(
                    out=out[nb * SQB + sq * P: nb * SQB + (sq + 1) * P,
                            nb2 * DH:(nb2 + 1) * DH],
                    in_=ob)
            return fn

        # ---- attention iteration bodies ----
        def it_of(i):
            nb, r = divmod(i, NHP * SKT)
            hp, sk = divmod(r, SKT)
            return nb, hp, sk

        ps_of = {}
        pr_of = {}
        pa_of = {}
        fillers = deque()

        def ready_sc(i):
            nb, hp, sk = it_of(i)
            return ((sk // 4, hp) in kg_done) and ((nb, hp) in qp_done)

        def pump():
            assert fillers, "filler deque empty while consumer not ready"
            fillers.popleft()()

        def emit_sc(i):
            nb, hp, sk = it_of(i)
            while not ready_sc(i):
                pump()
            ps = ps_big.tile([P, 2, DH], f32, tag="ps_big", name=f"sc{i}")
            q = qtiles[nb][hp]
            nc.tensor.matmul(
                ps[:, 0, :], lhsT=kT[hp][0:DK, sk * P:(sk + 1) * P],
                rhs=q[0:DK, :], start=True, stop=True)
            nc.tensor.matmul(
                ps[:, 1, :], lhsT=kT[hp][DK:P, sk * P:(sk + 1) * P],
                rhs=q[DK:P, :], start=True, stop=True)
            ps_of[i] = ps

        def emit_exp(i):
            ps = ps_of.pop(i)
            pr = probs_pool.tile([P, 2, DH], bf16, tag="probs", name=f"pr{i}")
            nc.scalar.activation(pr.rearrange("p a b -> p (a b)"),
                                 ps.rearrange("p a b -> p (a b)"),
                                 EXP, scale=0.125)
            pr_of[i] = pr

        def emit_pv(i):
            nb, hp, sk = it_of(i)
            while (sk // 4, sk % 4) not in vg_done:
                pump()
            if sk == 0:
                pa_e = ps_attn.tile([DK + 1, DH], f32, tag="ps_attn",
                                    name=f"pae{nb}_{hp}")
                pa_o = ps_attn.tile([DK + 1, DH], f32, tag="ps_attn",
                                    name=f"pao{nb}_{hp}")
                pa_of[(nb, hp)] = (pa_e, pa_o)
            pa_e, pa_o = pa_of[(nb, hp)]
            pr = pr_of.pop(i)
            nc.tensor.matmul(pa_e, lhsT=vA[sk][:, 2 * hp, :], rhs=pr[:, 0, :],
                             start=(sk == 0), stop=(sk == SKT - 1))
            nc.tensor.matmul(pa_o, lhsT=vA[sk][:, 2 * hp + 1, :], rhs=pr[:, 1, :],
                             start=(sk == 0), stop=(sk == SKT - 1))
            if sk == SKT - 1:
                # pair done: evict attn rows + softmax sums, queue its norm
                c3 = c3_pool.tile([2, SQB], f32, tag="c3", name=f"c3_{nb}_{hp}")
                raws = []
                for pa, h in ((pa_e, 2 * hp), (pa_o, 2 * hp + 1)):
                    raw = raw_pool.tile([DK + 1, SQB], f32, tag="raw",
                                        name=f"raw{nb}_{h}")
                    nc.vector.tensor_copy(raw, pa)
                    nc.sync.dma_start(out=c3[h % 2:h % 2 + 1, :],
                                      in_=raw[DK:DK + 1, :])
                    raws.append(raw)
                del pa_of[(nb, hp)]
                fillers.append(norm_pair(nb, hp, raws[0], raws[1], c3))

        # ---- prologue head: first projection groups inline ----
        kgroup(0, 0)()
        for j in range(SQB // P):
            vgroup(0, j)()
        qgroup(0, 0)()

        # block-0 filler schedule, ordered by first need
        fillers.extend([
            load_kv(1), kgroup(1, 0),
            vgroup(1, 0), vgroup(1, 1), vgroup(1, 2), vgroup(1, 3),
            qgroup(0, 1), kgroup(0, 1), kgroup(1, 1),
            load_kv(2), kgroup(2, 0),
            vgroup(2, 0), vgroup(2, 1), vgroup(2, 2), vgroup(2, 3),
            kgroup(2, 1),
            qgroup(0, 2), kgroup(0, 2), kgroup(1, 2), kgroup(2, 2),
            load_kv(3), kgroup(3, 0),
            vgroup(3, 0), vgroup(3, 1), vgroup(3, 2), vgroup(3, 3),
            kgroup(3, 1), kgroup(3, 2),
            qgroup(0, 3), kgroup(0, 3),
        ])
        fillers.append(qload(1))

        def late_weights():
            nc.gpsimd.dma_start(out=wo_sb, in_=wo[:, :, :, :])
        fillers.append(late_weights)
        fillers.extend([
            kgroup(1, 3), qgroup(1, 0), kgroup(2, 3), qgroup(1, 1),
            kgroup(3, 3), qgroup(1, 2), qgroup(1, 3),
        ])

        # ---- main software-pipelined loop ----
        emit_sc(0)
        emit_sc(1)
        for i in range(T):
            nb, hp, sk = it_of(i)
            if sk == 0 and hp == 0 and nb >= 1:
                # block boundary: queue prev block's out-proj + next q-proj
                for sq in range(SQB // P):
                    for nb2 in range(2):
                        fillers.append(outproj(nb - 1, sq, nb2))
                if nb + 1 < NB:
                    fillers.append(qload(nb + 1))
                    for ds in range(DST):
                        fillers.append(qgroup(nb + 1, ds))
            emit_exp(i)
            if i + 2 < T:
                emit_sc(i + 2)
            emit_pv(i)
            if i % 3 == 2 and fillers:
                fillers.popleft()()

        # ---- tail: drain remaining fillers, then last block's out-proj ----
        while fillers:
            fillers.popleft()()
        for sq in range(SQB // P):
            for nb2 in range(2):
                outproj(NB - 1, sq, nb2)()

    nc.compile()
    return nc


_NC_CACHE = {}


def _get_nc():
    if "nc" not in _NC_CACHE:
        _NC_CACHE["nc"] = build()
    return _NC_CACHE["nc"]


def _tile_xt(x):
    # [S, DM] -> transpose -> [DIT, NB, P, SQB] with each [P, SQB] contiguous
    xt = np.ascontiguousarray(x.T)                      # [DM, S]
    return np.ascontiguousarray(
        xt.reshape(DIT, P, NB, SQB).transpose(0, 2, 1, 3)).astype(BF)


def _wcol_dsmajor(W, cs):
    # [DM, DH-slice] -> [DST, P, DIT, P] (ds-major stationary layout)
    return np.ascontiguousarray(
        W[:, cs].reshape(DIT, P, DST, P).transpose(2, 1, 0, 3)).astype(BF)


def _shard_inputs(Q, K, V, Wq, bq, Wk, bk, Wv, bv, Wo, bo):
    in_maps = []
    qkvT = {}
    for b in range(4):
        qkvT[b] = (_tile_xt(Q[b]), _tile_xt(K[b]), _tile_xt(V[b]))
    halves = []
    for h in range(2):
        cs = slice(h * DH, (h + 1) * DH)
        halves.append(dict(
            wq=_wcol_dsmajor(Wq, cs),
            wk=_wcol_dsmajor(Wk, cs),
            wv=np.ascontiguousarray(
                Wv[:, cs].reshape(DIT, P, DH).transpose(1, 0, 2)).astype(BF),
            wo=np.ascontiguousarray(
                Wo[cs, :].reshape(NHP, P, 2, DH).transpose(1, 0, 2, 3)).astype(BF),
            bqt=np.ascontiguousarray(
                bq[cs].reshape(DST, P).T).astype(np.float32),
            bkt=np.ascontiguousarray(
                bk[cs].reshape(DST, P).T).astype(np.float32),
            bv=bv[cs].reshape(1, DH).astype(np.float32),
        ))
    for c in range(8):
        b, h = c // 2, c % 2
        qT, kT_, vT = qkvT[b]
        m = dict(qt=qT, kt=kT_, vt=vT)
        m.update(halves[h])
        in_maps.append(m)
    return in_maps


TRACE = False
LAST_RESULT = None


def kernel(**inputs):
    global LAST_RESULT
    inputs = {k: np.asarray(v, dtype=np.float32) for k, v in inputs.items()}
    nc = _get_nc()
    in_maps = _shard_inputs(
        inputs["Q"], inputs["K"], inputs["V"],
        inputs["Wq"], inputs["bq"], inputs["Wk"], inputs["bk"],
        inputs["Wv"], inputs["bv"], inputs["Wo"], inputs["bo"])
    r = run_bass_kernel_spmd(nc, in_maps, core_ids=list(range(8)), trace=TRACE)
    LAST_RESULT = r
    outs = [np.asarray(r.results[c]["out"], dtype=np.float32) for c in range(8)]
    full = np.stack([outs[2 * b] + outs[2 * b + 1] for b in range(4)], axis=0)
    return full + inputs["bo"].reshape(1, 1, DM)


# revision 20
# speedup vs baseline: 1.0843x; 1.0156x over previous
"""Multi-head attention (B=4, S=2048, D=1024, H=16, d_k=64) on 8 TRN2 NeuronCores.

Sharding: batch x head-half grid. Core c handles batch c//2 and head-half c%2
(8 of 16 heads). W_q/W_k/W_v are column-split, W_o row-split (tensor parallel);
the two partial outputs per batch are summed on the host (+bo also host-side).

All matmul operands are bf16 (fp32 PSUM accumulation). Bias adds ride the
PSUM->SBUF eviction on DVE (per-partition tensor_scalar for q/k whose bias
lies along partitions, broadcast tensor_tensor for v) instead of K=1 matmuls.

Emission is a software pipeline over 256 attention iterations (nb, hp, sk)
with lookahead 2: body(i) = [exp(i); scores(i+2); PV(i)] so the in-order PE
queue always has scores(i+2) issued BEFORE PV(i) stalls on exp(i) — ScalarE
exps then run back-to-back. K/V/Q projections, per-pair softmax
normalization (reciprocal + DRAM-bounce partition broadcast + DVE multiply)
and the out-projection of the previous block are pushed onto a filler deque
and popped between iterations (paced 1 per 3, plus a readiness pump that
force-pops producers before the consumer iteration is emitted). Block 0's
attention thereby streams directly behind the K/V projection prologue, and
block 3's normalization runs inside block 3 instead of as a serial tail.
"""

from collections import deque
from contextlib import ExitStack

import numpy as np
import ml_dtypes

import concourse.bass as bass
import concourse.mybir as mybir
import concourse.tile as tile
from concourse import bacc
from concourse.bass_utils import run_bass_kernel_spmd

P = 128
S = 2048
DM = 1024          # d_model
DH = 512           # per-core projected dim (8 heads x 64)
DK = 64
NH = 8             # heads per core
NHP = 4            # head pairs per core
SQB = 512          # Sq block width
NB = S // SQB      # 4 blocks
SKT = S // P       # 16 Sk tiles
DIT = DM // P      # 8 d_in tiles
DST = DH // P      # 4 d_out 128-slices (= head pairs)
T = NB * NHP * SKT # 256 attention iterations

f32 = mybir.dt.float32
bf16 = mybir.dt.bfloat16
EXP = mybir.ActivationFunctionType.Exp
ADD = mybir.AluOpType.add
MULT = mybir.AluOpType.mult
BF = ml_dtypes.bfloat16


def build():
    nc = bacc.Bacc("TRN2", target_bir_lowering=False, debug=False)

    qt = nc.declare_dram_parameter("qt", [DIT, NB, P, SQB], bf16, isOutput=False)
    kt = nc.declare_dram_parameter("kt", [DIT, NB, P, SQB], bf16, isOutput=False)
    vt = nc.declare_dram_parameter("vt", [DIT, NB, P, SQB], bf16, isOutput=False)
    # ds-major weight layouts: the ds=0 quarter arrives first so the first
    # projection group can start ~3.5us in.
    wq = nc.declare_dram_parameter("wq", [DST, P, DIT, P], bf16, isOutput=False)
    wk = nc.declare_dram_parameter("wk", [DST, P, DIT, P], bf16, isOutput=False)
    wv = nc.declare_dram_parameter("wv", [P, DIT, DH], bf16, isOutput=False)
    wo = nc.declare_dram_parameter("wo", [P, NHP, 2, DH], bf16, isOutput=False)
    bqt = nc.declare_dram_parameter("bqt", [P, DST], f32, isOutput=False)
    bkt = nc.declare_dram_parameter("bkt", [P, DST], f32, isOutput=False)
    bv = nc.declare_dram_parameter("bv", [1, DH], f32, isOutput=False)
    out = nc.declare_dram_parameter("out", [S, DM], f32, isOutput=True)

    scr = nc.dram_tensor("scr", [NB, NH, SQB], f32)

    with tile.TileContext(nc) as tc, ExitStack() as ctx:
        const = ctx.enter_context(tc.tile_pool(name="const", bufs=1))
        kT_pool = ctx.enter_context(tc.tile_pool(name="kT", bufs=1))
        vA_pool = ctx.enter_context(tc.tile_pool(name="vA", bufs=1))
        xin_pool = ctx.enter_context(tc.tile_pool(name="xin", bufs=10))
        wkv_pool = ctx.enter_context(tc.tile_pool(name="wkv", bufs=1))

        ps_mm = ctx.enter_context(tc.tile_pool(name="ps_mm", bufs=2, space="PSUM"))
        ps_big = ctx.enter_context(tc.tile_pool(name="ps_big", bufs=2, space="PSUM"))
        ps_attn = ctx.enter_context(tc.tile_pool(name="ps_attn", bufs=2, space="PSUM"))

        # ---- prologue-critical DMAs, most-urgent first ----
        wk_sb = wkv_pool.tile([P, DST, DIT, P], bf16)
        nc.gpsimd.dma_start(out=wk_sb[:, 0], in_=wk[0])

        def load_x(src, skb, tag, split=False):
            # kx readers (kgroup ds=0..3) are spread across all of block 0,
            # so kx tiles are fully resident (bufs=32, no slot reuse); vx
            # readers run right after their load -> 16 rotating slots.
            # split=True stripes the 8 loads across both trigger queues to
            # halve the cold-start latency of the very first tiles.
            ts = []
            eng = nc.sync if tag == "kx" else nc.gpsimd
            for di in range(DIT):
                e = (nc.sync if di % 2 == 0 else nc.gpsimd) if split else eng
                t = wkv_pool.tile([P, SQB], bf16, tag=tag,
                                  bufs=(32 if tag == "kx" else 16),
                                  name=f"{tag}{skb}_{di}")
                e.dma_start(out=t, in_=src[di, skb])
                ts.append(t)
            return ts

        kxs = {0: load_x(kt, 0, "kx", split=True)}
        wv_sb = wkv_pool.tile([P, DIT, DH], bf16)
        nc.gpsimd.dma_start(out=wv_sb[:, 0:DIT // 2, :], in_=wv[:, 0:DIT // 2, :])
        nc.gpsimd.dma_start(out=wv_sb[:, DIT // 2:, :], in_=wv[:, DIT // 2:, :])
        vxs = {0: load_x(vt, 0, "vx")}
        bkt_sb = const.tile([P, DST], f32)
        nc.sync.dma_start(out=bkt_sb, in_=bkt[:, :])
        bv_sb = const.tile([P, DH], f32)
        nc.sync.dma_start(out=bv_sb, in_=bv[0, :].partition_broadcast(P))
        ones128 = const.tile([P, NH], bf16)
        nc.vector.memset(ones128, 1.0)
        for i in range(1, DST):
            nc.gpsimd.dma_start(out=wk_sb[:, i], in_=wk[i])
        wq_sb = const.tile([P, DST, DIT, P], bf16)
        qx = {0: []}
        for di in range(DIT):
            t = xin_pool.tile([P, SQB], bf16, tag="xin", name=f"qx0_{di}")
            nc.sync.dma_start(out=t, in_=qt[di, 0])
            qx[0].append(t)
        nc.gpsimd.dma_start(out=wq_sb[:, 0], in_=wq[0])
        for i in range(1, DST):
            nc.gpsimd.dma_start(out=wq_sb[:, i], in_=wq[i])
        bqt_sb = const.tile([P, DST], f32)
        nc.sync.dma_start(out=bqt_sb, in_=bqt[:, :])

        kT = [kT_pool.tile([P, S], bf16, name=f"kT{i}", tag=f"kT{i}")
              for i in range(DST)]
        vA = [vA_pool.tile([P, NH, DK + 1], bf16, name=f"vA{i}", tag=f"vA{i}")
              for i in range(SKT)]

        # late-needed weights (queued behind the prologue stream)
        wo_sb = const.tile([P, NHP, 2, DH], bf16)
        bo_loaded = [False]

        # ---- emitted-state tracking for the scheduler ----
        kg_done = set()    # (skb, ds)
        vg_done = set()    # (skb, j)
        qp_done = set()    # (nb, ds)
        qtiles = {nb: [None] * DST for nb in range(NB)}
        pairs = {nb: [None] * NHP for nb in range(NB)}

        def kgroup(skb, ds):
            def fn():
                ps = ps_mm.tile([P, DH], f32, tag="ps_mm", name=f"psk{skb}_{ds}")
                for di in range(DIT):
                    nc.tensor.matmul(
                        ps, lhsT=wk_sb[:, ds, di, :], rhs=kxs[skb][di],
                        start=(di == 0), stop=(di == DIT - 1))
                nc.vector.tensor_scalar_add(
                    kT[ds][:, skb * SQB:(skb + 1) * SQB], ps,
                    bkt_sb[:, ds:ds + 1])
                kg_done.add((skb, ds))
            return fn

        def vgroup(skb, j):
            def fn():
                skt = skb * (SQB // P) + j
                ps = ps_mm.tile([P, DH], f32, tag="ps_mm", name=f"psv{skb}_{j}")
                for di in range(DIT):
                    nc.tensor.matmul(
                        ps, lhsT=vxs[skb][di][:, j * P:(j + 1) * P],
                        rhs=wv_sb[:, di, :],
                        start=(di == 0), stop=(di == DIT - 1))
                va = vA[skt]
                nc.vector.tensor_copy(va[:, :, DK], ones128)
                nc.vector.tensor_tensor(
                    va[:, :, 0:DK], ps.rearrange("p (h x) -> p h x", x=DK),
                    bv_sb.rearrange("p (h x) -> p h x", x=DK), ADD)
                vg_done.add((skb, j))
            return fn

        def load_kv(skb):
            def fn():
                kxs[skb] = load_x(kt, skb, "kx")
                vxs[skb] = load_x(vt, skb, "vx")
            return fn

        def qload(nb):
            def fn():
                qx[nb] = []
                for di in range(DIT):
                    t = xin_pool.tile([P, SQB], bf16, tag="xin",
                                      name=f"qx{nb}_{di}")
                    nc.sync.dma_start(out=t, in_=qt[di, nb])
                    qx[nb].append(t)
            return fn

        def qgroup(nb, ds):
            def fn():
                ps = ps_mm.tile([P, DH], f32, tag="ps_mm", name=f"psq{nb}_{ds}")
                for di in range(DIT):
                    nc.tensor.matmul(
                        ps, lhsT=wq_sb[:, ds, di, :], rhs=qx[nb][di],
                        start=(di == 0), stop=(di == DIT - 1))
                qtile = qT_pool.tile([P, SQB], bf16, tag="qT", name=f"qT{nb}_{ds}")
                nc.vector.tensor_scalar_add(qtile, ps, bqt_sb[:, ds:ds + 1])
                qtiles[nb][ds] = qtile
                qp_done.add((nb, ds))
            return fn

        qT_pool = ctx.enter_context(tc.tile_pool(name="qT", bufs=8))
        probs_pool = ctx.enter_context(tc.tile_pool(name="probs", bufs=3))
        raw_pool = ctx.enter_context(tc.tile_pool(name="raw", bufs=6))
        pair_pool = ctx.enter_context(tc.tile_pool(name="pair", bufs=8))
        ostg_pool = ctx.enter_context(tc.tile_pool(name="ostg", bufs=2))
        c3_pool = ctx.enter_context(tc.tile_pool(name="c3", bufs=4))
        bc_pool = ctx.enter_context(tc.tile_pool(name="bc", bufs=2))
        ob_pool = ctx.enter_context(tc.tile_pool(name="ob", bufs=4))

        def norm_pair(nb, hp, raw_e, raw_o, c3):
            """Fast reciprocal of the pair's softmax sums, DRAM bounce,
            partition broadcast, DVE normalize into the [128,512] bf16
            pair tile."""
            def fn():
                nc.vector.reciprocal_approx_fast(c3, c3)
                nc.sync.dma_start(out=scr[nb, 2 * hp:2 * hp + 2, :], in_=c3)
                pair = pair_pool.tile([P, SQB], bf16, tag="pair",
                                      name=f"pair{nb}_{hp}")
                pairs[nb][hp] = pair
                bce = bc_pool.tile([DK, SQB], f32, tag="bc", name=f"bce{nb}_{hp}")
                nc.sync.dma_start(
                    out=bce, in_=scr[nb, 2 * hp, :].partition_broadcast(DK))
                nc.vector.tensor_mul(pair[0:DK, :], raw_e[0:DK, :], bce)
                bco = bc_pool.tile([DK, SQB], f32, tag="bc", name=f"bco{nb}_{hp}")
                nc.sync.dma_start(
                    out=bco, in_=scr[nb, 2 * hp + 1, :].partition_broadcast(DK))
                ostg = ostg_pool.tile([DK, SQB], bf16, tag="ostg",
                                      name=f"ostg{nb}_{hp}")
                nc.vector.tensor_mul(ostg, raw_o[0:DK, :], bco)
                nc.sync.dma_start(out=pair[DK:P, :], in_=ostg)
            return fn

        def outproj(nb, sq, nb2):
            def fn():
                if not bo_loaded[0]:
                    bo_loaded[0] = True
                pso = ps_mm.tile([P, DH], f32, tag="ps_mm",
                                 name=f"pso{nb}_{sq}_{nb2}")
                for hp in range(NHP):
                    nc.tensor.matmul(
                        pso, lhsT=pairs[nb][hp][:, sq * P:(sq + 1) * P],
                        rhs=wo_sb[:, hp, nb2, :],
                        start=(hp == 0), stop=(hp == NHP - 1))
                ob = ob_pool.tile([P, DH], f32, tag="ob", name=f"ob{nb}_{sq}_{nb2}")
                nc.vector.tensor_copy(ob, pso)
                eng = nc.gpsimd if (sq + nb2) % 2 == 0 else nc.sync
                eng.dma_start(
                    out=out[nb * SQB + sq * P: nb * SQB + (sq + 1) * P,
                            nb2 * DH:(nb2 + 1) * DH],
                    in_=ob)
            return fn

        # ---- attention iteration bodies ----
        def it_of(i):
            nb, r = divmod(i, NHP * SKT)
            hp, sk = divmod(r, SKT)
            return nb, hp, sk

        ps_of = {}
        pr_of = {}
        pa_of = {}
        fillers = deque()

        def ready_sc(i):
            nb, hp, sk = it_of(i)
            return ((sk // 4, hp) in kg_done) and ((nb, hp) in qp_done)

        def pump():
            assert fillers, "filler deque empty while consumer not ready"
            fillers.popleft()()

        def emit_sc(i):
            nb, hp, sk = it_of(i)
            while not ready_sc(i):
                pump()
            ps = ps_big.tile([P, 2, DH], f32, tag="ps_big", name=f"sc{i}")
            q = qtiles[nb][hp]
            nc.tensor.matmul(
                ps[:, 0, :], lhsT=kT[hp][0:DK, sk * P:(sk + 1) * P],
                rhs=q[0:DK, :], start=True, stop=True)
            nc.tensor.matmul(
                ps[:, 1, :], lhsT=kT[hp][DK:P, sk * P:(sk + 1) * P],
                rhs=q[DK:P, :], start=True, stop=True)
            ps_of[i] = ps

        def emit_exp(i):
            ps = ps_of.pop(i)
            pr = probs_pool.tile([P, 2, DH], bf16, tag="probs", name=f"pr{i}")
            nc.scalar.activation(pr.rearrange("p a b -> p (a b)"),
                                 ps.rearrange("p a b -> p (a b)"),
                                 EXP, scale=0.125)
            pr_of[i] = pr

        def emit_pv(i):
            nb, hp, sk = it_of(i)
            while (sk // 4, sk % 4) not in vg_done:
                pump()
            if sk == 0:
                pa_e = ps_attn.tile([DK + 1, DH], f32, tag="ps_attn",
                                    name=f"pae{nb}_{hp}")
                pa_o = ps_attn.tile([DK + 1, DH], f32, tag="ps_attn",
                                    name=f"pao{nb}_{hp}")
                pa_of[(nb, hp)] = (pa_e, pa_o)
            pa_e, pa_o = pa_of[(nb, hp)]
            pr = pr_of.pop(i)
            nc.tensor.matmul(pa_e, lhsT=vA[sk][:, 2 * hp, :], rhs=pr[:, 0, :],
                             start=(sk == 0), stop=(sk == SKT - 1))
            nc.tensor.matmul(pa_o, lhsT=vA[sk][:, 2 * hp + 1, :], rhs=pr[:, 1, :],
                             start=(sk == 0), stop=(sk == SKT - 1))
            if sk == SKT - 1:
                # pair done: evict attn rows + softmax sums, queue its norm
                c3 = c3_pool.tile([2, SQB], f32, tag="c3", name=f"c3_{nb}_{hp}")
                raws = []
                for pa, h in ((pa_e, 2 * hp), (pa_o, 2 * hp + 1)):
                    raw = raw_pool.tile([DK + 1, SQB], f32, tag="raw",
                                        name=f"raw{nb}_{h}")
                    nc.vector.tensor_copy(raw, pa)
                    nc.sync.dma_start(out=c3[h % 2:h % 2 + 1, :],
                                      in_=raw[DK:DK + 1, :])
                    raws.append(raw)
                del pa_of[(nb, hp)]
                fillers.append(norm_pair(nb, hp, raws[0], raws[1], c3))

        # ---- prologue head: first projection groups inline ----
        kgroup(0, 0)()
        for j in range(SQB // P):
            vgroup(0, j)()
        qgroup(0, 0)()

        # block-0 filler schedule, ordered by first need
        fillers.extend([
            load_kv(1), kgroup(1, 0),
            vgroup(1, 0), vgroup(1, 1), vgroup(1, 2), vgroup(1, 3),
            qgroup(0, 1), kgroup(0, 1), kgroup(1, 1),
            load_kv(2), kgroup(2, 0),
            vgroup(2, 0), vgroup(2, 1), vgroup(2, 2), vgroup(2, 3),
            kgroup(2, 1),
            qgroup(0, 2), kgroup(0, 2), kgroup(1, 2), kgroup(2, 2),
            load_kv(3), kgroup(3, 0),
            vgroup(3, 0), vgroup(3, 1), vgroup(3, 2), vgroup(3, 3),
            kgroup(3, 1), kgroup(3, 2),
            qgroup(0, 3), kgroup(0, 3),
        ])
        fillers.append(qload(1))

        def late_weights():
            nc.gpsimd.dma_start(out=wo_sb, in_=wo[:, :, :, :])
        fillers.append(late_weights)
        fillers.extend([
            kgroup(1, 3), qgroup(1, 0), kgroup(2, 3), qgroup(1, 1),
            kgroup(3, 3), qgroup(1, 2), qgroup(1, 3),
        ])

        # ---- main software-pipelined loop ----
        emit_sc(0)
        emit_sc(1)
        for i in range(T):
            nb, hp, sk = it_of(i)
            if sk == 0 and hp == 0 and nb >= 1:
                # block boundary: queue prev block's out-proj + next q-proj
                for sq in range(SQB // P):
                    for nb2 in range(2):
                        fillers.append(outproj(nb - 1, sq, nb2))
                if nb + 1 < NB:
                    fillers.append(qload(nb + 1))
                    for ds in range(DST):
                        fillers.append(qgroup(nb + 1, ds))
            emit_exp(i)
            if i + 2 < T:
                emit_sc(i + 2)
            emit_pv(i)
            if i % 3 == 2 and fillers:
                fillers.popleft()()

        # ---- tail: drain remaining fillers, then last block's out-proj ----
        while fillers:
            fillers.popleft()()
        for sq in range(SQB // P):
            for nb2 in range(2):
                outproj(NB - 1, sq, nb2)()

    nc.compile()
    return nc


_NC_CACHE = {}


def _get_nc():
    if "nc" not in _NC_CACHE:
        _NC_CACHE["nc"] = build()
    return _NC_CACHE["nc"]


def _tile_xt(x):
    # [S, DM] -> transpose -> [DIT, NB, P, SQB] with each [P, SQB] contiguous
    xt = np.ascontiguousarray(x.T)                      # [DM, S]
    return np.ascontiguousarray(
        xt.reshape(DIT, P, NB, SQB).transpose(0, 2, 1, 3)).astype(BF)


def _wcol_dsmajor(W, cs):
    # [DM, DH-slice] -> [DST, P, DIT, P] (ds-major stationary layout)
    return np.ascontiguousarray(
        W[:, cs].reshape(DIT, P, DST, P).transpose(2, 1, 0, 3)).astype(BF)


def _shard_inputs(Q, K, V, Wq, bq, Wk, bk, Wv, bv, Wo, bo):
    in_maps = []
    qkvT = {}
    for b in range(4):
        qkvT[b] = (_tile_xt(Q[b]), _tile_xt(K[b]), _tile_xt(V[b]))
    halves = []
    for h in range(2):
        cs = slice(h * DH, (h + 1) * DH)
        halves.append(dict(
            wq=_wcol_dsmajor(Wq, cs),
            wk=_wcol_dsmajor(Wk, cs),
            wv=np.ascontiguousarray(
                Wv[:, cs].reshape(DIT, P, DH).transpose(1, 0, 2)).astype(BF),
            wo=np.ascontiguousarray(
                Wo[cs, :].reshape(NHP, P, 2, DH).transpose(1, 0, 2, 3)).astype(BF),
            bqt=np.ascontiguousarray(
                bq[cs].reshape(DST, P).T).astype(np.float32),
            bkt=np.ascontiguousarray(
                bk[cs].reshape(DST, P).T).astype(np.float32),
            bv=bv[cs].reshape(1, DH).astype(np.float32),
        ))
    for c in range(8):
        b, h = c // 2, c % 2
        qT, kT_, vT = qkvT[b]
        m = dict(qt=qT, kt=kT_, vt=vT)
        m.update(halves[h])
        in_maps.append(m)
    return in_maps


TRACE = False
LAST_RESULT = None


def kernel(**inputs):
    global LAST_RESULT
    inputs = {k: np.asarray(v, dtype=np.float32) for k, v in inputs.items()}
    nc = _get_nc()
    in_maps = _shard_inputs(
        inputs["Q"], inputs["K"], inputs["V"],
        inputs["Wq"], inputs["bq"], inputs["Wk"], inputs["bk"],
        inputs["Wv"], inputs["bv"], inputs["Wo"], inputs["bo"])
    r = run_bass_kernel_spmd(nc, in_maps, core_ids=list(range(8)), trace=TRACE)
    LAST_RESULT = r
    outs = [np.asarray(r.results[c]["out"], dtype=np.float32) for c in range(8)]
    full = np.stack([outs[2 * b] + outs[2 * b + 1] for b in range(4)], axis=0)
    return full + inputs["bo"].reshape(1, 1, DM)
